# revision 5
# baseline (speedup 1.0000x reference)
"""XNOR/ReActNet binarized 3x3 conv on 8 Trainium2 NeuronCores.

out = conv2d(sign(x - alpha), sign(weight), stride 1, pad 1)
  x      [32, 256, 56, 56] f32
  alpha  [256, 1, 1]       f32
  weight [256, 256, 3, 3]  f32
  out    [32, 256, 56, 56] f32

Strategy (data-parallel): each core takes 4 images. Binarized values are
exactly +-1, so they are exact in fp8e4; the conv runs as 9 shifted
matmuls (one per kernel tap) in fp8 DoubleRow mode (contraction over all
256 input channels per matmul: 128 partitions x 2 k-tiles), accumulating
in fp32 PSUM. All sums are small integers -> bit-exact vs the reference.

v2 schedule changes (vs v1 at 131.2us):
  - PE warm-up: a block of dummy 128x128 matmuls at t=0 keeps the PE HAM
    activity monitor busy through the initial DMA window, so the real
    transposes/matmuls run at 2.4 GHz instead of 1.2 GHz.
  - weight DMA split per ci-chunk ([128,1152] x2 per co) and triggered
    from the Activation HWDGE so sign/transpose pipeline starts earlier.
  - x DMAs merged: one dma_start per (img, row-range) covering both ci
    chunks ([p (n h w)] AP); image 0 split in 3 row segments (10/26/26
    padded rows) so the first conv matmuls only wait on a 0.5MB DMA.
  - output DMA: one trigger per (img,co) group (gathered [128,3136]),
    except the last group which drains per-sp-chunk to keep the tail
    short. Fewer triggers -> less serialization on the sync sequencer
    (~0.6us per DIRECT2D trigger).
"""

import numpy as np

import concourse.bass as bass
import concourse.mybir as mybir
import concourse.tile as tile
from concourse.masks import make_identity
from concourse.bass_utils import run_bass_kernel_spmd

N_CORES = 8
B, C, H, W, KS = 32, 256, 56, 56, 3
BL = B // N_CORES           # images per core
PH, PW = H + 2, 64          # padded rows, row stride (58 x 64)
NPIX = H * W                # 3136
RPC = 8                     # output rows per PSUM tile
NSP = H // RPC              # 7 spatial chunks
NFREE = RPC * W             # 448 (fits one 2KB f32 PSUM bank)
NCH = C // 128              # 2 channel chunks
HKK = (C // NCH) * KS * KS  # 1152 weight cols per ci chunk
F32 = mybir.dt.float32
BF16 = mybir.dt.bfloat16
FP8 = mybir.dt.float8e4
SIGN = mybir.ActivationFunctionType.Sign
DR = mybir.MatmulPerfMode.DoubleRow

import os as _os
N_WARM = int(_os.environ.get("K_WARM", "130"))


def _split_excess_waits(nc):
    """This walrus build rejects instructions carrying more than one sem
    wait ("Too many sync wait commands" from setupSyncWait). Tile's
    scheduler can attach several. Hoist the excess onto same-engine NoOps
    placed just before the instruction - engines are in-order, so the
    semantics are identical."""
    k = 0
    for f in nc.m.functions:
        for bb in f.blocks:
            old = list(bb.instructions)
            new = []
            changed = False
            for ins in old:
                si = ins.sync_info
                waits = list(si.on_wait) if si and si.on_wait else []
                if len(waits) > 1:
                    for w in waits[:-1]:
                        nop = mybir.InstNoOp(
                            name=f"I-wsplit{k}",
                            ins=[],
                            outs=[],
                            engine=ins.engine,
                            sync_info=mybir.SyncInfo(on_wait=[w], on_update=[]),
                        )
                        k += 1
                        new.append(nop)
                    si.on_wait = waits[-1:]
                    changed = True
                new.append(ins)
            if changed:
                bb.instructions[:] = new


def _build_program() -> bass.Bass:
    nc = bass.Bass()
    x = nc.dram_tensor("x", [BL, C, H, W], F32, kind="ExternalInput")
    alpha = nc.dram_tensor("alpha", [C], F32, kind="ExternalInput")
    weight = nc.dram_tensor("weight", [C, C, KS, KS], F32, kind="ExternalInput")
    out = nc.dram_tensor("out", [BL, C, H, W], F32, kind="ExternalOutput")

    wv = weight[:].rearrange("o i kh kw -> o (i kh kw)")
    ov = out[:].rearrange("b c h w -> b c (h w)")
    # x viewed as [img][p, ci_chunk, h, w] with c = n*128 + p
    xim = [
        x[img].rearrange("(n p) h w -> p n h w", p=128)
        for img in range(BL)
    ]

    # image row segments: (seg_name, padded_row_start, padded_row_end,
    #                      x_row_start, n_x_rows, zero_top, zero_bot)
    # img0 is split 4 ways so the first conv matmuls wait on ~1MB of DMA
    # instead of 3.2MB; the rest of the rows stream in behind the PE.
    SEGS0 = [
        ("a", 0, 18, 0, 17, True, False),    # sp0-1
        ("b", 16, 34, 15, 18, False, False),  # sp2-3
        ("c", 32, 50, 31, 18, False, False),  # sp4-5
        ("d", 48, 58, 47, 9, False, True),    # sp6
    ]
    SEGS = [
        ("a", 0, 34, 0, 33, True, False),   # sp0-3
        ("c", 32, 58, 31, 25, False, True),  # sp4-6
    ]
    XSMAX = NCH * 33 * W  # staging tile cols (largest segment)

    with tile.TileContext(nc) as tc:
        with (
            tc.tile_pool(name="const", bufs=1) as constp,
            tc.tile_pool(name="apad", bufs=1) as apadp,
            tc.tile_pool(name="wsb", bufs=1) as wsbp,
            tc.tile_pool(name="xs", bufs=3) as xsp,
            tc.tile_pool(name="outs", bufs=2) as outsp,
        ):
            ident = constp.tile([128, 128], BF16, tag="ident")
            make_identity(nc, ident[:])

            alpha_sb = constp.tile([128, NCH], F32, tag="alpha")
            nc.sync.dma_start(alpha_sb[:], alpha[:].rearrange("(n p) -> p n", p=128))

            # Binarized transposed weights packed for DoubleRow:
            # wdr[(kh,kw,co)] = fp8 [128 ci_local, 2 ci_chunk, 128 co]
            wdr = {}
            for co in range(NCH):
                for kh in range(KS):
                    for kw in range(KS):
                        wdr[(kh, kw, co)] = constp.tile(
                            [128, NCH * 128], FP8,
                            tag=f"wdr{co}_{kh}_{kw}",
                            name=f"wdr{co}_{kh}_{kw}",
                        )

            def warmup(psc):
                """Dummy matmuls: keep the PE busy during the initial DMA
                window so HAM un-throttles to 2.4 GHz before real work.
                Reuses the conv PSUM tag so no extra banks are allocated."""
                if N_WARM <= 0:
                    return
                wt = psc.tile([128, NFREE], F32, tag="conv", name="warm")
                for i in range(N_WARM):
                    nc.tensor.matmul(wt[:, :128], ident[:], ident[:],
                                     start=True, stop=True)

            def prep_weights_dma(co, eng):
                tiles = []
                for ci in range(NCH):
                    wraw = wsbp.tile([128, HKK], F32, tag=f"wraw{co}_{ci}",
                                     name=f"wraw{co}_{ci}")
                    eng.dma_start(
                        wraw[:],
                        wv[co * 128:(co + 1) * 128,
                           ci * HKK:(ci + 1) * HKK],
                    )
                    tiles.append(wraw)
                return tiles

            def prep_weights_sign(co, wraws):
                wbins = []
                for ci in range(NCH):
                    wbin = wsbp.tile([128, HKK], BF16, tag=f"wbin{co}_{ci}",
                                     name=f"wbin{co}_{ci}")
                    nc.scalar.activation(wbin[:], wraws[ci][:], SIGN)
                    wbins.append(wbin)
                return wbins

            def prep_weights_tr(co, wbins, pswt):
                for ci in range(NCH):
                    wb3 = wbins[ci][:].rearrange("p (c k) -> p c k", c=128)
                    for kh in range(KS):
                        for kw in range(KS):
                            pt = pswt.tile([128, 128], BF16, tag="tp")
                            nc.tensor.transpose(
                                pt[:], wb3[:, :, kh * KS + kw], ident[:]
                            )
                            nc.vector.tensor_copy(
                                wdr[(kh, kw, co)][:, ci * 128:(ci + 1) * 128],
                                pt[:],
                            )

            neg_alpha = constp.tile([128, NCH], F32, tag="nalpha")

            # apad[img] = list of (a4 view, g0, g1) row segments
            apad = {}

            def prep_image(img, mid=None):
                segs = SEGS0 if img == 0 else SEGS
                out_segs = []
                for si, (sn, g0, g1, xr0, nxr, ztop, zbot) in enumerate(segs):
                    if mid is not None and si == len(segs) - 1:
                        mid()  # co1 weight DMA triggers land before the
                        # last (sp6) segment in HBM queue order
                        mid = None
                    nr = g1 - g0
                    t = apadp.tile([128, NCH * nr * PW], FP8,
                                   tag=f"ap{sn}{img}", name=f"ap{sn}{img}")
                    a4 = t[:].rearrange("p (c h w) -> p c h w", c=NCH, h=nr)
                    for ci in range(NCH):
                        if ztop:
                            nc.gpsimd.memset(a4[:, ci, 0, :], 0.0)
                        if zbot:
                            nc.gpsimd.memset(a4[:, ci, nr - 1, :], 0.0)
                        nc.gpsimd.memset(a4[:, ci, :, 0], 0.0)
                        nc.gpsimd.memset(a4[:, ci, :, W + 1], 0.0)
                    xs_t = xsp.tile([128, XSMAX], F32, tag="xs")
                    nc.sync.dma_start(
                        xs_t[:, :NCH * nxr * W],
                        xim[img][:, :, xr0:xr0 + nxr, :],
                    )
                    xs4 = xs_t[:, :NCH * nxr * W].rearrange(
                        "p (c h w) -> p c h w", c=NCH, h=nxr
                    )
                    lo = 1 if ztop else 0
                    for ci in range(NCH):
                        nc.scalar.activation(
                            a4[:, ci, lo:lo + nxr, 1:W + 1],
                            xs4[:, ci],
                            SIGN,
                            bias=neg_alpha[:, ci:ci + 1],
                        )
                    out_segs.append((a4, g0, g1))
                apad[img] = out_segs

            # Conv: per spatial chunk, 9 DoubleRow matmuls (one per tap)
            # accumulated in PSUM, DVE-drained into a gathered [128,3136]
            # tile; one output DMA per group (per-sp for the last group).
            n_acc = KS * KS

            def conv_group(img, co, psc, last=False):
                segs = apad[img]
                ot = outsp.tile([128, NPIX], F32, tag="out",
                                name=f"ot{img}_{co}")
                for sp in range(NSP):
                    pt = psc.tile([128, NFREE], F32, tag="conv",
                                  name=f"pt{img}_{co}_{sp}")
                    i_acc = 0
                    for kh in range(KS):
                        g = sp * RPC + kh
                        for (a4, g0, g1) in segs:
                            if g >= g0 and g + RPC <= g1:
                                break
                        else:
                            raise AssertionError((img, sp, kh))
                        r0 = g - g0
                        for kw in range(KS):
                            w3 = wdr[(kh, kw, co)][:].rearrange(
                                "p (c m) -> p c m", c=NCH
                            )
                            rhs = a4[:, :, r0:r0 + RPC, kw:kw + W]
                            nc.tensor.matmul(
                                pt[:], w3, rhs,
                                start=i_acc == 0,
                                stop=i_acc == n_acc - 1,
                                perf_mode=DR,
                            )
                            i_acc += 1
                    nc.vector.tensor_copy(
                        ot[:, sp * NFREE:(sp + 1) * NFREE], pt[:]
                    )
                    if last:
                        nc.sync.dma_start(
                            ov[img, co * 128:(co + 1) * 128,
                               sp * NFREE:(sp + 1) * NFREE],
                            ot[:, sp * NFREE:(sp + 1) * NFREE],
                        )
                if not last:
                    nc.sync.dma_start(
                        ov[img, co * 128:(co + 1) * 128, :],
                        ot[:],
                    )

            # Emission order: warmup dummies fill the PE during the DMA
            # window; co1's transposes are deferred until after the first
            # conv group. PSUM: 2 transpose banks + 6 conv banks = 8.
            with (
                tc.tile_pool(name="pswt", bufs=2, space="PSUM") as pswt,
                tc.tile_pool(name="psc", bufs=6, space="PSUM") as psc,
            ):
                warmup(psc)
                w0 = prep_weights_dma(0, nc.scalar)
                nc.scalar.mul(neg_alpha[:], alpha_sb[:], -1.0)
                wb0 = prep_weights_sign(0, w0)
                w1 = []
                prep_image(0, mid=lambda: w1.extend(
                    prep_weights_dma(1, nc.sync)))
                prep_weights_tr(0, wb0, pswt)
                conv_group(0, 0, psc)
                wb1 = prep_weights_sign(1, w1)
                prep_weights_tr(1, wb1, pswt)
                prep_image(1)
                conv_group(0, 1, psc)
                conv_group(1, 0, psc)
                prep_image(2)
                conv_group(1, 1, psc)
                conv_group(2, 0, psc)
                prep_image(3)
                conv_group(2, 1, psc)
                conv_group(3, 0, psc)
                conv_group(3, 1, psc, last=True)
    _split_excess_waits(nc)
    return nc


_prog_cache = {}


def _get_program() -> bass.Bass:
    if "nc" not in _prog_cache:
        _prog_cache["nc"] = _build_program()
    return _prog_cache["nc"]


def _run(x, alpha, weight, trace=False):
    x = np.ascontiguousarray(np.asarray(x, dtype=np.float32))
    alpha = np.ascontiguousarray(np.asarray(alpha, dtype=np.float32).reshape(C))
    weight = np.ascontiguousarray(np.asarray(weight, dtype=np.float32))
    assert x.shape == (B, C, H, W) and weight.shape == (C, C, KS, KS)

    nc = _get_program()
    in_maps = [
        {
            "x": np.ascontiguousarray(x[i * BL:(i + 1) * BL]),
            "alpha": alpha,
            "weight": weight,
        }
        for i in range(N_CORES)
    ]
    res = run_bass_kernel_spmd(nc, in_maps, list(range(N_CORES)), trace=trace)
    out = np.concatenate([res.results[i]["out"] for i in range(N_CORES)], axis=0)
    return out.astype(np.float32, copy=False), res


def kernel(x, alpha, weight):
    out, _ = _run(x, alpha, weight, trace=False)
    return out


def kernel_timed(x, alpha, weight):
    out, res = _run(x, alpha, weight, trace=True)
    return out, res


# revision 6
# speedup vs baseline: 1.0361x; 1.0361x over previous
"""XNOR/ReActNet binarized 3x3 conv on 8 Trainium2 NeuronCores.

out = conv2d(sign(x - alpha), sign(weight), stride 1, pad 1)
  x      [32, 256, 56, 56] f32
  alpha  [256, 1, 1]       f32
  weight [256, 256, 3, 3]  f32
  out    [32, 256, 56, 56] f32

Strategy (data-parallel): each core takes 4 images. Binarized values are
exactly +-1, so they are exact in fp8e4; the conv runs as 9 shifted
matmuls (one per kernel tap) in fp8 DoubleRow mode (contraction over all
256 input channels per matmul: 128 partitions x 2 k-tiles), accumulating
in fp32 PSUM. All sums are small integers -> bit-exact vs the reference.

v2 schedule changes (vs v1 at 131.2us):
  - PE warm-up: a block of dummy 128x128 matmuls at t=0 keeps the PE HAM
    activity monitor busy through the initial DMA window, so the real
    transposes/matmuls run at 2.4 GHz instead of 1.2 GHz.
  - weight DMA split per ci-chunk ([128,1152] x2 per co) and triggered
    from the Activation HWDGE so sign/transpose pipeline starts earlier.
  - x DMAs merged: one dma_start per (img, row-range) covering both ci
    chunks ([p (n h w)] AP); image 0 split in 3 row segments (10/26/26
    padded rows) so the first conv matmuls only wait on a 0.5MB DMA.
  - output DMA: one trigger per (img,co) group (gathered [128,3136]),
    except the last group which drains per-sp-chunk to keep the tail
    short. Fewer triggers -> less serialization on the sync sequencer
    (~0.6us per DIRECT2D trigger).
"""

import numpy as np

import concourse.bass as bass
import concourse.mybir as mybir
import concourse.tile as tile
from concourse.masks import make_identity
from concourse.bass_utils import run_bass_kernel_spmd

N_CORES = 8
B, C, H, W, KS = 32, 256, 56, 56, 3
BL = B // N_CORES           # images per core
PH, PW = H + 2, 64          # padded rows, row stride (58 x 64)
NPIX = H * W                # 3136
RPC = 8                     # output rows per PSUM tile
NSP = H // RPC              # 7 spatial chunks
NFREE = RPC * W             # 448 (fits one 2KB f32 PSUM bank)
NCH = C // 128              # 2 channel chunks
HKK = (C // NCH) * KS * KS  # 1152 weight cols per ci chunk
F32 = mybir.dt.float32
BF16 = mybir.dt.bfloat16
FP8 = mybir.dt.float8e4
SIGN = mybir.ActivationFunctionType.Sign
DR = mybir.MatmulPerfMode.DoubleRow

import os as _os
N_WARM = int(_os.environ.get("K_WARM", "40"))


def _split_excess_waits(nc):
    """This walrus build rejects instructions carrying more than one sem
    wait ("Too many sync wait commands" from setupSyncWait). Tile's
    scheduler can attach several. Hoist the excess onto same-engine NoOps
    placed just before the instruction - engines are in-order, so the
    semantics are identical."""
    k = 0
    for f in nc.m.functions:
        for bb in f.blocks:
            old = list(bb.instructions)
            new = []
            changed = False
            for ins in old:
                si = ins.sync_info
                waits = list(si.on_wait) if si and si.on_wait else []
                if len(waits) > 1:
                    for w in waits[:-1]:
                        nop = mybir.InstNoOp(
                            name=f"I-wsplit{k}",
                            ins=[],
                            outs=[],
                            engine=ins.engine,
                            sync_info=mybir.SyncInfo(on_wait=[w], on_update=[]),
                        )
                        k += 1
                        new.append(nop)
                    si.on_wait = waits[-1:]
                    changed = True
                new.append(ins)
            if changed:
                bb.instructions[:] = new


def _build_program() -> bass.Bass:
    nc = bass.Bass()
    x = nc.dram_tensor("x", [BL, C, H, W], F32, kind="ExternalInput")
    alpha = nc.dram_tensor("alpha", [C], F32, kind="ExternalInput")
    weight = nc.dram_tensor("weight", [C, C, KS, KS], F32, kind="ExternalInput")
    out = nc.dram_tensor("out", [BL, C, H, W], F32, kind="ExternalOutput")

    wv = weight[:].rearrange("o i kh kw -> o (i kh kw)")
    ov = out[:].rearrange("b c h w -> b c (h w)")
    # x viewed as [img][p, ci_chunk, h, w] with c = n*128 + p
    xim = [
        x[img].rearrange("(n p) h w -> p n h w", p=128)
        for img in range(BL)
    ]

    # image row segments: (seg_name, padded_row_start, padded_row_end,
    #                      x_row_start, n_x_rows, zero_top, zero_bot)
    # img0 is split 4 ways so the first conv matmuls wait on ~1MB of DMA
    # instead of 3.2MB; the rest of the rows stream in behind the PE.
    SEGS0 = [
        ("a", 0, 18, 0, 17, True, False),    # sp0-1
        ("b", 16, 34, 15, 18, False, False),  # sp2-3
        ("c", 32, 50, 31, 18, False, False),  # sp4-5
        ("d", 48, 58, 47, 9, False, True),    # sp6
    ]
    SEGS = [
        ("a", 0, 34, 0, 33, True, False),   # sp0-3
        ("c", 32, 58, 31, 25, False, True),  # sp4-6
    ]
    XSMAX = NCH * 33 * W  # staging tile cols (largest segment)

    with tile.TileContext(nc) as tc:
        with (
            tc.tile_pool(name="const", bufs=1) as constp,
            tc.tile_pool(name="apad", bufs=1) as apadp,
            tc.tile_pool(name="wsb", bufs=1) as wsbp,
            tc.tile_pool(name="xs", bufs=3) as xsp,
            tc.tile_pool(name="outs", bufs=2) as outsp,
        ):
            ident = constp.tile([128, 128], BF16, tag="ident")
            make_identity(nc, ident[:])

            alpha_sb = constp.tile([128, NCH], F32, tag="alpha")
            nc.sync.dma_start(alpha_sb[:], alpha[:].rearrange("(n p) -> p n", p=128))

            # Binarized transposed weights packed for DoubleRow:
            # wdr[(kh,kw,co)] = fp8 [128 ci_local, 2 ci_chunk, 128 co]
            wdr = {}
            for co in range(NCH):
                for kh in range(KS):
                    for kw in range(KS):
                        wdr[(kh, kw, co)] = constp.tile(
                            [128, NCH * 128], FP8,
                            tag=f"wdr{co}_{kh}_{kw}",
                            name=f"wdr{co}_{kh}_{kw}",
                        )

            def warmup(psc):
                """Dummy matmuls: keep the PE busy during the initial DMA
                window so HAM un-throttles to 2.4 GHz before real work.
                Reuses the conv PSUM tag so no extra banks are allocated."""
                if N_WARM <= 0:
                    return
                wt = psc.tile([128, NFREE], F32, tag="conv", name="warm")
                for i in range(N_WARM):
                    nc.tensor.matmul(wt[:, :128], ident[:], ident[:],
                                     start=True, stop=True)

            def prep_weights_dma(co, eng):
                tiles = []
                for ci in range(NCH):
                    wraw = wsbp.tile([128, HKK], F32, tag=f"wraw{co}_{ci}",
                                     name=f"wraw{co}_{ci}")
                    eng.dma_start(
                        wraw[:],
                        wv[co * 128:(co + 1) * 128,
                           ci * HKK:(ci + 1) * HKK],
                    )
                    tiles.append(wraw)
                return tiles

            def prep_weights_sign(co, wraws):
                wbins = []
                for ci in range(NCH):
                    wbin = wsbp.tile([128, HKK], BF16, tag=f"wbin{co}_{ci}",
                                     name=f"wbin{co}_{ci}")
                    nc.scalar.activation(wbin[:], wraws[ci][:], SIGN)
                    wbins.append(wbin)
                return wbins

            def prep_weights_sign_one(co, ci, wraw):
                wbin = wsbp.tile([128, HKK], BF16, tag=f"wbin{co}_{ci}",
                                 name=f"wbin{co}_{ci}")
                nc.scalar.activation(wbin[:], wraw[:], SIGN)
                return wbin

            def prep_weights_tr_ci(co, ci, wbin, pswt):
                wb3 = wbin[:].rearrange("p (c k) -> p c k", c=128)
                for kh in range(KS):
                    for kw in range(KS):
                        pt = pswt.tile([128, 128], BF16, tag="tp")
                        nc.tensor.transpose(
                            pt[:], wb3[:, :, kh * KS + kw], ident[:]
                        )
                        nc.vector.tensor_copy(
                            wdr[(kh, kw, co)][:, ci * 128:(ci + 1) * 128],
                            pt[:],
                        )

            def prep_weights_tr(co, wbins, pswt):
                for ci in range(NCH):
                    wb3 = wbins[ci][:].rearrange("p (c k) -> p c k", c=128)
                    for kh in range(KS):
                        for kw in range(KS):
                            pt = pswt.tile([128, 128], BF16, tag="tp")
                            nc.tensor.transpose(
                                pt[:], wb3[:, :, kh * KS + kw], ident[:]
                            )
                            nc.vector.tensor_copy(
                                wdr[(kh, kw, co)][:, ci * 128:(ci + 1) * 128],
                                pt[:],
                            )

            neg_alpha = constp.tile([128, NCH], F32, tag="nalpha")

            # apad[img] = list of (a4 view, g0, g1) row segments
            apad = {}

            def prep_seg_dma(img, segdef):
                (sn, g0, g1, xr0, nxr, ztop, zbot) = segdef
                nr = g1 - g0
                t = apadp.tile([128, NCH * nr * PW], FP8,
                               tag=f"ap{sn}{img}", name=f"ap{sn}{img}")
                a4 = t[:].rearrange("p (c h w) -> p c h w", c=NCH, h=nr)
                for ci in range(NCH):
                    if ztop:
                        nc.gpsimd.memset(a4[:, ci, 0, :], 0.0)
                    if zbot:
                        nc.gpsimd.memset(a4[:, ci, nr - 1, :], 0.0)
                    nc.gpsimd.memset(a4[:, ci, :, 0], 0.0)
                    nc.gpsimd.memset(a4[:, ci, :, W + 1], 0.0)
                xs_t = xsp.tile([128, XSMAX], F32, tag="xs")
                nc.sync.dma_start(
                    xs_t[:, :NCH * nxr * W],
                    xim[img][:, :, xr0:xr0 + nxr, :],
                )
                return (a4, g0, g1, xs_t, nxr, ztop)

            def prep_seg_sign(info):
                (a4, g0, g1, xs_t, nxr, ztop) = info
                xs4 = xs_t[:, :NCH * nxr * W].rearrange(
                    "p (c h w) -> p c h w", c=NCH, h=nxr
                )
                lo = 1 if ztop else 0
                for ci in range(NCH):
                    nc.scalar.activation(
                        a4[:, ci, lo:lo + nxr, 1:W + 1],
                        xs4[:, ci],
                        SIGN,
                        bias=neg_alpha[:, ci:ci + 1],
                    )
                return (a4, g0, g1)

            def prep_image(img):
                infos = [prep_seg_dma(img, sd) for sd in SEGS]
                apad[img] = [prep_seg_sign(i) for i in infos]

            # Conv: per spatial chunk, 9 DoubleRow matmuls (one per tap)
            # accumulated in PSUM, DVE-drained into a gathered [128,3136]
            # tile; one output DMA per group (per-sp for the last group).
            n_acc = KS * KS

            def conv_group(img, co, psc, drain="group"):
                segs = apad[img]
                ot = outsp.tile([128, NPIX], F32, tag="out",
                                name=f"ot{img}_{co}")
                for sp in range(NSP):
                    pt = psc.tile([128, NFREE], F32, tag="conv",
                                  name=f"pt{img}_{co}_{sp}")
                    i_acc = 0
                    for kh in range(KS):
                        g = sp * RPC + kh
                        for (a4, g0, g1) in segs:
                            if g >= g0 and g + RPC <= g1:
                                break
                        else:
                            raise AssertionError((img, sp, kh))
                        r0 = g - g0
                        for kw in range(KS):
                            w3 = wdr[(kh, kw, co)][:].rearrange(
                                "p (c m) -> p c m", c=NCH
                            )
                            rhs = a4[:, :, r0:r0 + RPC, kw:kw + W]
                            nc.tensor.matmul(
                                pt[:], w3, rhs,
                                start=i_acc == 0,
                                stop=i_acc == n_acc - 1,
                                perf_mode=DR,
                            )
                            i_acc += 1
                    nc.vector.tensor_copy(
                        ot[:, sp * NFREE:(sp + 1) * NFREE], pt[:]
                    )
                    if drain == "sp" or (drain == "half" and sp in (3, 6)):
                        c0 = 0 if sp <= 3 and drain == "half" else (
                            4 * NFREE if drain == "half" else sp * NFREE)
                        c1 = (sp + 1) * NFREE
                        nc.sync.dma_start(
                            ov[img, co * 128:(co + 1) * 128, c0:c1],
                            ot[:, c0:c1],
                        )
                if drain == "group":
                    nc.sync.dma_start(
                        ov[img, co * 128:(co + 1) * 128, :],
                        ot[:],
                    )

            # Emission order: warmup dummies fill the PE during the DMA
            # window; co1's transposes are deferred until after the first
            # conv group. PSUM: 2 transpose banks + 6 conv banks = 8.
            with (
                tc.tile_pool(name="pswt", bufs=2, space="PSUM") as pswt,
                tc.tile_pool(name="psc", bufs=6, space="PSUM") as psc,
            ):
                warmup(psc)
                # DMA queue order: w-ci0, seg-a, w-ci1, seg-b/c/d, w-co1.
                # The first transposes need only w-ci0; seg-a jumps ahead
                # of w-ci1 so the first conv matmuls start ~2us earlier.
                wraw00 = wsbp.tile([128, HKK], F32, tag="wraw0_0",
                                   name="wraw0_0")
                nc.scalar.dma_start(wraw00[:], wv[0:128, 0:HKK])
                nc.scalar.mul(neg_alpha[:], alpha_sb[:], -1.0)
                sa = prep_seg_dma(0, SEGS0[0])
                wraw01 = wsbp.tile([128, HKK], F32, tag="wraw0_1",
                                   name="wraw0_1")
                nc.sync.dma_start(wraw01[:], wv[0:128, HKK:2 * HKK])
                sb = prep_seg_dma(0, SEGS0[1])
                sc = prep_seg_dma(0, SEGS0[2])
                sd = prep_seg_dma(0, SEGS0[3])
                w1 = prep_weights_dma(1, nc.sync)
                # scalar order: sign w-ci0, sign seg-a, sign w-ci1, rest
                wb0 = prep_weights_sign_one(0, 0, wraw00)
                seg_a = prep_seg_sign(sa)
                wb1_ = prep_weights_sign_one(0, 1, wraw01)
                apad[0] = [seg_a, prep_seg_sign(sb), prep_seg_sign(sc),
                           prep_seg_sign(sd)]
                prep_weights_tr_ci(0, 0, wb0, pswt)
                prep_weights_tr_ci(0, 1, wb1_, pswt)
                conv_group(0, 0, psc)
                wbs1 = prep_weights_sign(1, w1)
                prep_weights_tr(1, wbs1, pswt)
                prep_image(1)
                conv_group(0, 1, psc)
                conv_group(1, 0, psc)
                prep_image(2)
                conv_group(1, 1, psc)
                conv_group(2, 0, psc)
                prep_image(3)
                conv_group(2, 1, psc)
                conv_group(3, 0, psc, drain="half")
                conv_group(3, 1, psc, drain="sp")
    _split_excess_waits(nc)
    return nc


_prog_cache = {}


def _get_program() -> bass.Bass:
    if "nc" not in _prog_cache:
        _prog_cache["nc"] = _build_program()
    return _prog_cache["nc"]


def _run(x, alpha, weight, trace=False):
    x = np.ascontiguousarray(np.asarray(x, dtype=np.float32))
    alpha = np.ascontiguousarray(np.asarray(alpha, dtype=np.float32).reshape(C))
    weight = np.ascontiguousarray(np.asarray(weight, dtype=np.float32))
    assert x.shape == (B, C, H, W) and weight.shape == (C, C, KS, KS)

    nc = _get_program()
    in_maps = [
        {
            "x": np.ascontiguousarray(x[i * BL:(i + 1) * BL]),
            "alpha": alpha,
            "weight": weight,
        }
        for i in range(N_CORES)
    ]
    res = run_bass_kernel_spmd(nc, in_maps, list(range(N_CORES)), trace=trace)
    out = np.concatenate([res.results[i]["out"] for i in range(N_CORES)], axis=0)
    return out.astype(np.float32, copy=False), res


def kernel(x, alpha, weight):
    out, _ = _run(x, alpha, weight, trace=False)
    return out


def kernel_timed(x, alpha, weight):
    out, res = _run(x, alpha, weight, trace=True)
    return out, res
